# revision 7
# baseline (speedup 1.0000x reference)
"""DeepONet+GRU Trainium2 kernel (8-core data parallel).

Full inputs in, full outputs out. Batch 1024 is sharded 128/core across 8
NeuronCores; all parameters are replicated. Per core:
  branch MLP (528->512 x4, transposed layout) -> branchT [512f, 128b]
  trunk MLP (1->512 x4) -> trunkT [512f, 128t]
  seq = branchT.T @ trunkT (+don_bias folded into GRU aug weights)
  2-layer GRU scan over T=128 steps, hidden 256, fully on-chip
  proj -> [128, 16]
"""
import sys
sys.path.insert(0, '/opt/trn_rl_repo')

import numpy as np

B = 1024
BC = 128          # batch per core
NB = 528
NBP = 640         # padded branch input (5 k-tiles)
HID = 512
GH = 256
T = 128
NS = 16
NCORES = 8

_CACHE = {}


def _patched_tile_context(nc):
    """TileContext whose tail drain splits sem waits (walrus CoreV3 rejects
    >1 sync wait on a Drain)."""
    import concourse.tile as tile
    from concourse.vector_clock import ScopedClock

    class PatchedTileContext(tile.TileContext):
        def _drain_and_barrier(self, tick_clock, wait_clock):
            nc = self.nc
            drain_inst = nc.sync.drain()
            wait_clock.add_sem_waits(
                drain_inst.ins, ScopedClock({None: tick_clock.global_clock})
            )
            si = drain_inst.ins.sync_info
            waits = list(si.on_wait or []) if si is not None else []
            if len(waits) > 1:
                si.on_wait = waits[:1]
                for i in range(1, len(waits)):
                    extra = nc.sync.drain()
                    esi = extra.ins.sync_info
                    if esi is None:
                        from concourse import mybir
                        extra.ins.sync_info = mybir.SyncInfo(
                            on_wait=waits[i:i + 1], on_update=[]
                        )
                    else:
                        esi.on_wait = waits[i:i + 1]
            nc.all_engine_barrier()
            assert self.sems is not None
            popped = nc._tile_sem_poison_stack.pop()
            assert popped is self._sem_poison
            nc.clear_and_free_semaphores(list(self.sems.allocated().values()))
            nc.all_engine_barrier()

    return PatchedTileContext(nc)


def _split_multi_waits(nc):
    """This container's walrus rejects >1 sync wait per instruction
    ("Too many sync wait commands"). Hoist extra waits onto engine-matched
    NoOps spliced immediately before the offending instruction."""
    from concourse import mybir
    n_extra = 0
    for fn in nc.m.functions:
        for bb in fn.blocks:
            new = []
            for inst in bb.instructions:
                si = inst.sync_info
                waits = list(si.on_wait) if (si is not None and si.on_wait) else []
                if len(waits) > 1:
                    for w in waits[:-1]:
                        nop = mybir.InstNoOp(
                            name=f"wsplit-{n_extra}-{inst.name}",
                            engine=inst.engine,
                            bass_nofuse=True,
                            sync_info=mybir.SyncInfo(on_wait=[w], on_update=[]),
                        )
                        new.append(nop)
                        n_extra += 1
                    si.on_wait = [waits[-1]]
                new.append(inst)
            if n_extra:
                bb.instructions[:] = new
    return n_extra


def build_nc(n_steps=T):
    import concourse.bass as bass
    from concourse import mybir
    from contextlib import ExitStack

    FP = mybir.dt.float32
    AF = mybir.ActivationFunctionType
    nc = bass.Bass()

    # ---- DRAM parameters (host-prepped layouts) ----
    dp = lambda name, shape: nc.declare_dram_parameter(name, list(shape), FP, isOutput=False)
    xT_d = dp("xT", (5, 128, BC))
    bW_d = [dp("bW0", (5, 128, HID))] + [dp(f"bW{i}", (4, 128, HID)) for i in (1, 2, 3)]
    bb_d = [dp(f"bb{i}", (128, 4)) for i in range(4)]
    tW0_d = dp("tW0", (1, HID))
    tW_d = [None] + [dp(f"tW{i}", (4, 128, HID)) for i in (1, 2, 3)]
    tb_d = [dp(f"tb{i}", (128, 4)) for i in range(4)]
    tT_d = dp("tT", (1, T))
    whh0_d = dp("whh0", (2, 128, 3 * GH))
    whh1_d = dp("whh1", (2, 128, 3 * GH))
    wih1_d = dp("wih1", (2, 128, 3 * GH))
    a0rz_d = dp("a0rz", (2, 512))
    a0x_d = dp("a0x", (2, 512))
    b1rz_d = dp("b1rz", (1, 512))
    b1x_d = dp("b1x", (1, 512))
    pW_d = dp("pW", (2, 128, NS))
    pb_d = dp("pb", (1, NS))
    ident_d = dp("ident", (128, 128))
    ones16k_d = dp("ones16k", (1, T * BC))
    out_d = nc.declare_dram_parameter("out", [BC, NS], FP, isOutput=True)

    with ExitStack() as ctx:
        tc = ctx.enter_context(_patched_tile_context(nc))
        const = ctx.enter_context(tc.tile_pool(name="const", bufs=1))

        # ---- persistent SBUF ----
        ident = const.tile([128, 128], FP)
        nc.gpsimd.dma_start(ident[:], ident_d[:])
        whh0 = const.tile([128, 2 * 768], FP)
        whh1 = const.tile([128, 2 * 768], FP)
        wih1 = const.tile([128, 2 * 768], FP)
        for k in range(2):
            nc.gpsimd.dma_start(whh0[:, k * 768:(k + 1) * 768], whh0_d[k])
            nc.gpsimd.dma_start(whh1[:, k * 768:(k + 1) * 768], whh1_d[k])
            nc.gpsimd.dma_start(wih1[:, k * 768:(k + 1) * 768], wih1_d[k])
        a0rz = const.tile([2, 512], FP)
        nc.gpsimd.dma_start(a0rz[:], a0rz_d[:])
        a0x = const.tile([2, 512], FP)
        nc.gpsimd.dma_start(a0x[:], a0x_d[:])
        b1rz = const.tile([1, 512], FP)
        nc.gpsimd.dma_start(b1rz[:], b1rz_d[:])
        b1x = const.tile([1, 512], FP)
        nc.gpsimd.dma_start(b1x[:], b1x_d[:])
        pW = const.tile([128, 2 * NS], FP)
        for k in range(2):
            nc.gpsimd.dma_start(pW[:, k * NS:(k + 1) * NS], pW_d[k])
        pb = const.tile([1, NS], FP)
        nc.gpsimd.dma_start(pb[:], pb_d[:])
        ones1 = const.tile([1, 128], FP)
        nc.vector.memset(ones1[:], 1.0)

        branchT = const.tile([128, HID], FP)   # [feat within tile, 4 mtiles * batch]
        trunkT = const.tile([128, HID], FP)
        seqT_sb = const.tile([T, BC], FP)

        # states
        h0 = const.tile([128, GH], FP)
        h1 = const.tile([128, GH], FP)
        h0T = const.tile([128, GH], FP)
        h1T = const.tile([128, GH], FP)
        nc.vector.memset(h0[:], 0.0)
        nc.vector.memset(h1[:], 0.0)
        nc.vector.memset(h0T[:], 0.0)
        nc.vector.memset(h1T[:], 0.0)

        # ================= MLP phase =================
        with tc.tile_pool(name="mlpw", bufs=1) as mlpw, \
             tc.tile_pool(name="mlps", bufs=2) as mlps, \
             tc.tile_pool(name="mlpp", bufs=4, space=bass.MemorySpace.PSUM) as mlpp:

            def mlp(xtiles_sb, nk_first, W_sbs, b_sbs, final_relu, out_sb):
                # xtiles_sb: input tiles tensor [128, nk_first*128]
                cur = xtiles_sb
                nlayers = 4
                for l in range(nlayers):
                    nk = nk_first if l == 0 else 4
                    Wl = W_sbs[l]
                    dst = out_sb if l == nlayers - 1 else mlps.tile([128, HID], FP, tag="mlpact")
                    for m in range(4):
                        ps = mlpp.tile([128, 128], FP, tag="mlppsum")
                        for k in range(nk):
                            nc.tensor.matmul(
                                ps[:],
                                Wl[:, k * HID + m * 128: k * HID + (m + 1) * 128],
                                cur[:, k * 128:(k + 1) * 128],
                                start=(k == 0), stop=(k == nk - 1),
                            )
                        func = AF.Relu if (l < nlayers - 1 or final_relu) else AF.Identity
                        nc.scalar.activation(
                            dst[:, m * 128:(m + 1) * 128], ps[:], func,
                            bias=b_sbs[l][:, m:m + 1],
                        )
                    cur = dst
                return cur

            # branch weights -> SBUF
            bW_sb = []
            for l in range(4):
                nk = 5 if l == 0 else 4
                w = mlpw.tile([128, nk * HID], FP, tag=f"bw{l}")
                for k in range(nk):
                    nc.gpsimd.dma_start(w[:, k * HID:(k + 1) * HID], bW_d[l][k])
                bW_sb.append(w)
            bb_sb = []
            for l in range(4):
                t_ = mlpw.tile([128, 4], FP, tag=f"bb{l}")
                nc.gpsimd.dma_start(t_[:], bb_d[l][:])
                bb_sb.append(t_)
            xk = mlpw.tile([128, 5 * 128], FP, tag="xk")
            for k in range(5):
                nc.gpsimd.dma_start(xk[:, k * 128:(k + 1) * 128], xT_d[k])
            mlp(xk, 5, bW_sb, bb_sb, final_relu=False, out_sb=branchT)

            # trunk: first layer K=1
            tW0 = mlpw.tile([1, HID], FP, tag="tw0")
            nc.gpsimd.dma_start(tW0[:], tW0_d[:])
            tTs = mlpw.tile([1, T], FP, tag="tts")
            nc.gpsimd.dma_start(tTs[:], tT_d[:])
            tb_sb = []
            for l in range(4):
                t_ = mlpw.tile([128, 4], FP, tag=f"tb{l}")
                nc.gpsimd.dma_start(t_[:], tb_d[l][:])
                tb_sb.append(t_)
            tW_sb = [None]
            for l in (1, 2, 3):
                w = mlpw.tile([128, 4 * HID], FP, tag=f"tw{l}")
                for k in range(4):
                    nc.gpsimd.dma_start(w[:, k * HID:(k + 1) * HID], tW_d[l][k])
                tW_sb.append(w)

            tact = mlps.tile([128, HID], FP, tag="mlpact")
            for m in range(4):
                ps = mlpp.tile([128, 128], FP, tag="mlppsum")
                nc.tensor.matmul(ps[:], tW0[:, m * 128:(m + 1) * 128], tTs[:],
                                 start=True, stop=True)
                nc.scalar.activation(tact[:, m * 128:(m + 1) * 128], ps[:],
                                     AF.Relu, bias=tb_sb[0][:, m:m + 1])
            # layers 1..3 of trunk
            cur = tact
            for l in (1, 2, 3):
                dst = trunkT if l == 3 else mlps.tile([128, HID], FP, tag="mlpact")
                for m in range(4):
                    ps = mlpp.tile([128, 128], FP, tag="mlppsum")
                    for k in range(4):
                        nc.tensor.matmul(
                            ps[:],
                            tW_sb[l][:, k * HID + m * 128: k * HID + (m + 1) * 128],
                            cur[:, k * 128:(k + 1) * 128],
                            start=(k == 0), stop=(k == 3),
                        )
                    nc.scalar.activation(dst[:, m * 128:(m + 1) * 128], ps[:],
                                         AF.Relu, bias=tb_sb[l][:, m:m + 1])
                cur = dst

            # seq[b,t] = sum_f branchT[f,b] * trunkT[f,t]  -> [B, T] psum
            ps_seq = mlpp.tile([128, 128], FP, tag="mlppsum")
            for k in range(4):
                nc.tensor.matmul(ps_seq[:], branchT[:, k * 128:(k + 1) * 128],
                                 trunkT[:, k * 128:(k + 1) * 128],
                                 start=(k == 0), stop=(k == 3))
            seq_sb = mlps.tile([128, 128], FP, tag="seqsb")
            nc.scalar.copy(seq_sb[:], ps_seq[:])
            ps_seqT = mlpp.tile([128, 128], FP, tag="mlppsum")
            nc.tensor.transpose(ps_seqT[:], seq_sb[:], ident[:])
            nc.scalar.copy(seqT_sb[:], ps_seqT[:])

        # ================= GRU phase =================
        saug = const.tile([2, T * BC], FP)
        # partition-collapse seqT (t-major rows) into row 0 of saug
        nc.gpsimd.dma_start(saug[0:1, :], seqT_sb[:])
        nc.gpsimd.dma_start(saug[1:2, :], ones16k_d[:])

        with tc.tile_pool(name="gp2", bufs=1, space=bass.MemorySpace.PSUM) as gp2, \
             tc.tile_pool(name="gp1", bufs=1, space=bass.MemorySpace.PSUM) as gp1, \
             tc.tile_pool(name="gs", bufs=2) as gs:

            for t in range(n_steps):
                st = saug[:, t * BC:(t + 1) * BC]
                # ---- L0 matmuls ----
                P0rz = gp2.tile([128, 512], FP, tag="P0rz")
                nc.tensor.matmul(P0rz[:], h0T[:, 0:128], whh0[:, 0:512], start=True, stop=False)
                nc.tensor.matmul(P0rz[:], h0T[:, 128:256], whh0[:, 768:1280], start=False, stop=False)
                nc.tensor.matmul(P0rz[:], st, a0rz[:], start=False, stop=True)
                P0x = gp1.tile([128, 512], FP, tag="P0x")
                nc.tensor.matmul(P0x[:, 0:256], h0T[:, 0:128], whh0[:, 512:768], start=True, stop=False)
                nc.tensor.matmul(P0x[:, 0:256], h0T[:, 128:256], whh0[:, 1280:1536], start=False, stop=False)
                nc.tensor.matmul(P0x[:, 0:256], st, a0x[:, 0:256], start=False, stop=True)
                nc.tensor.matmul(P0x[:, 256:512], st, a0x[:, 256:512], start=True, stop=True)
                # ---- L0 gates ----
                rz0 = gs.tile([128, 512], FP, tag="rz0")
                nc.scalar.activation(rz0[:], P0rz[:], AF.Sigmoid)
                t1 = gs.tile([128, 256], FP, tag="t1")
                nc.vector.tensor_mul(t1[:], rz0[:, 0:256], P0x[:, 0:256])
                t2 = gs.tile([128, 256], FP, tag="t2")
                nc.vector.tensor_add(t2[:], t1[:], P0x[:, 256:512])
                n0 = gs.tile([128, 256], FP, tag="n0")
                nc.scalar.activation(n0[:], t2[:], AF.Tanh)
                d0 = gs.tile([128, 256], FP, tag="d0")
                nc.gpsimd.tensor_sub(d0[:], h0[:], n0[:])
                m0 = gs.tile([128, 256], FP, tag="m0")
                nc.vector.tensor_mul(m0[:], rz0[:, 256:512], d0[:])
                nc.vector.tensor_add(h0[:], n0[:], m0[:])
                # ---- transpose h0 ----
                Ptr0 = gp1.tile([128, 256], FP, tag="Ptr0")
                nc.tensor.transpose(Ptr0[:, 0:128], h0[:, 0:128], ident[:])
                nc.tensor.transpose(Ptr0[:, 128:256], h0[:, 128:256], ident[:])
                nc.scalar.copy(h0T[:], Ptr0[:])
                # ---- L1 matmuls ----
                P1rz = gp2.tile([128, 512], FP, tag="P1rz")
                nc.tensor.matmul(P1rz[:], h1T[:, 0:128], whh1[:, 0:512], start=True, stop=False)
                nc.tensor.matmul(P1rz[:], h1T[:, 128:256], whh1[:, 768:1280], start=False, stop=False)
                nc.tensor.matmul(P1rz[:], h0T[:, 0:128], wih1[:, 0:512], start=False, stop=False)
                nc.tensor.matmul(P1rz[:], h0T[:, 128:256], wih1[:, 768:1280], start=False, stop=False)
                nc.tensor.matmul(P1rz[:], ones1[:], b1rz[:], start=False, stop=True)
                P1x = gp1.tile([128, 512], FP, tag="P1x")
                nc.tensor.matmul(P1x[:, 0:256], h1T[:, 0:128], whh1[:, 512:768], start=True, stop=False)
                nc.tensor.matmul(P1x[:, 0:256], h1T[:, 128:256], whh1[:, 1280:1536], start=False, stop=False)
                nc.tensor.matmul(P1x[:, 0:256], ones1[:], b1x[:, 0:256], start=False, stop=True)
                nc.tensor.matmul(P1x[:, 256:512], h0T[:, 0:128], wih1[:, 512:768], start=True, stop=False)
                nc.tensor.matmul(P1x[:, 256:512], h0T[:, 128:256], wih1[:, 1280:1536], start=False, stop=False)
                nc.tensor.matmul(P1x[:, 256:512], ones1[:], b1x[:, 256:512], start=False, stop=True)
                # ---- L1 gates ----
                rz1 = gs.tile([128, 512], FP, tag="rz1")
                nc.scalar.activation(rz1[:], P1rz[:], AF.Sigmoid)
                t11 = gs.tile([128, 256], FP, tag="t11")
                nc.vector.tensor_mul(t11[:], rz1[:, 0:256], P1x[:, 0:256])
                t21 = gs.tile([128, 256], FP, tag="t21")
                nc.vector.tensor_add(t21[:], t11[:], P1x[:, 256:512])
                n1 = gs.tile([128, 256], FP, tag="n1")
                nc.scalar.activation(n1[:], t21[:], AF.Tanh)
                d1 = gs.tile([128, 256], FP, tag="d1")
                nc.gpsimd.tensor_sub(d1[:], h1[:], n1[:])
                m1 = gs.tile([128, 256], FP, tag="m1")
                nc.vector.tensor_mul(m1[:], rz1[:, 256:512], d1[:])
                nc.vector.tensor_add(h1[:], n1[:], m1[:])
                # ---- transpose h1 ----
                Ptr1 = gp1.tile([128, 256], FP, tag="Ptr1")
                nc.tensor.transpose(Ptr1[:, 0:128], h1[:, 0:128], ident[:])
                nc.tensor.transpose(Ptr1[:, 128:256], h1[:, 128:256], ident[:])
                nc.scalar.copy(h1T[:], Ptr1[:])

            # ---- projection ----
            Pout = gp1.tile([128, NS], FP, tag="Pout")
            nc.tensor.matmul(Pout[:], h1T[:, 0:128], pW[:, 0:NS], start=True, stop=False)
            nc.tensor.matmul(Pout[:], h1T[:, 128:256], pW[:, NS:2 * NS], start=False, stop=False)
            nc.tensor.matmul(Pout[:], ones1[:], pb[:], start=False, stop=True)
            out_sb = gs.tile([128, NS], FP, tag="outsb")
            nc.scalar.copy(out_sb[:], Pout[:])
            nc.gpsimd.dma_start(out_d[:], out_sb[:])

    _split_multi_waits(nc)
    return nc


def prep_inputs(inputs):
    """Host-side shared (per-core-identical) tensor prep."""
    f = np.float32
    g = {}
    bWf = np.asarray(inputs['branch_Wf'], f)      # (512, 528)
    bWr = np.asarray(inputs['branch_Wr'], f)      # (3, 512, 512)
    w = np.zeros((NBP, HID), f)
    w[:NB] = bWf.T
    g['bW0'] = w.reshape(5, 128, HID)
    for i in range(3):
        g[f'bW{i + 1}'] = np.ascontiguousarray(bWr[i].T).reshape(4, 128, HID)
    g['bb0'] = np.asarray(inputs['branch_bf'], f).reshape(4, 128).T.copy()
    for i in range(3):
        g[f'bb{i + 1}'] = np.asarray(inputs['branch_br'][i], f).reshape(4, 128).T.copy()
    g['tW0'] = np.asarray(inputs['trunk_Wf'], f).T.copy()          # (1, 512)
    tWr = np.asarray(inputs['trunk_Wr'], f)
    for i in range(3):
        g[f'tW{i + 1}'] = np.ascontiguousarray(tWr[i].T).reshape(4, 128, HID)
    g['tb0'] = np.asarray(inputs['trunk_bf'], f).reshape(4, 128).T.copy()
    for i in range(3):
        g[f'tb{i + 1}'] = np.asarray(inputs['trunk_br'][i], f).reshape(4, 128).T.copy()
    g['tT'] = np.arange(T, dtype=f).reshape(1, T)
    g['whh0'] = np.ascontiguousarray(np.asarray(inputs['gru_Whh0'], f).T).reshape(2, 128, 768)
    g['whh1'] = np.ascontiguousarray(np.asarray(inputs['gru_Whh1'], f).T).reshape(2, 128, 768)
    g['wih1'] = np.ascontiguousarray(np.asarray(inputs['gru_Wih1'], f).T).reshape(2, 128, 768)
    don = float(np.asarray(inputs['don_bias'], f).reshape(-1)[0])
    w0 = np.asarray(inputs['gru_Wih0'], f)[:, 0]  # (768,)
    bih0 = np.asarray(inputs['gru_bih0'], f)
    bhh0 = np.asarray(inputs['gru_bhh0'], f)
    a0rz = np.zeros((2, 512), f)
    a0rz[0] = w0[:512]
    a0rz[1] = bih0[:512] + bhh0[:512] + don * w0[:512]
    g['a0rz'] = a0rz
    a0x = np.zeros((2, 512), f)
    a0x[0, 0:256] = 0.0
    a0x[1, 0:256] = bhh0[512:768]
    a0x[0, 256:512] = w0[512:768]
    a0x[1, 256:512] = bih0[512:768] + don * w0[512:768]
    g['a0x'] = a0x
    bih1 = np.asarray(inputs['gru_bih1'], f)
    bhh1 = np.asarray(inputs['gru_bhh1'], f)
    g['b1rz'] = (bih1[:512] + bhh1[:512]).reshape(1, 512).copy()
    b1x = np.zeros((1, 512), f)
    b1x[0, 0:256] = bhh1[512:768]
    b1x[0, 256:512] = bih1[512:768]
    g['b1x'] = b1x
    g['pW'] = np.ascontiguousarray(np.asarray(inputs['proj_W'], f).T).reshape(2, 128, NS)
    g['pb'] = np.asarray(inputs['proj_b'], f).reshape(1, NS)
    g['ident'] = np.eye(128, dtype=f)
    g['ones16k'] = np.ones((1, T * BC), f)
    return g


def run(inputs, **spmd_kwargs):
    from concourse.bass_utils import run_bass_kernel_spmd

    if 'nc' not in _CACHE:
        _CACHE['nc'] = build_nc(T)
    nc = _CACHE['nc']

    shared = prep_inputs(inputs)
    x = np.asarray(inputs['x'], np.float32)
    in_maps = []
    for c in range(NCORES):
        xs = x[c * BC:(c + 1) * BC]          # (128, 528)
        xt = np.zeros((NBP, BC), np.float32)
        xt[:NB] = xs.T
        m = dict(shared)
        m['xT'] = xt.reshape(5, 128, BC)
        in_maps.append(m)

    res = run_bass_kernel_spmd(nc, in_maps, list(range(NCORES)), **spmd_kwargs)
    out = np.concatenate([res.results[c]["out"] for c in range(NCORES)], axis=0)
    return out.astype(np.float32), res


def kernel(**inputs):
    out, _ = run(inputs)
    return out


if __name__ == "__main__":
    rng = np.random.RandomState(0)
    print("building nc...")
    nc = build_nc(2)
    print("built OK")
